# revision 1
# baseline (speedup 1.0000x reference)
"""Kalman CV filter (nn_KalmanCV) — Trainium2 Bass kernel, 8-core data parallel.

Math: the covariance P (and thus the Kalman gains K_t and the output
channels sx/sy/rho) is batch-independent — it depends only on the scalar
inputs. The whole per-batch computation therefore collapses to a linear
map over the 32 history scalars:

    out[l, b, ch<2] = sum_{t,ci} W[t*2+ci, l*5+ch] * hist[t, b, ci]
    out[l, b, ch>=2] = const[l, ch]          (sx, sy, rho)

Device kernel per core: tiled matmul (32x125)^T @ (32x512) on the PE with
a per-partition bias add (the constant channels) on the ScalarE, then DMA
out. Batch is sharded across 8 cores; layout transforms happen host-side.
"""
import numpy as np

DT = 0.2
LEN_HIST = 16
LEN_PRED = 25
BATCH = 100000

N_CORES = 8
NB = 512                    # batch columns per matmul tile
NTILES = 25                 # tiles per core
BS = NB * NTILES            # 12800 padded batch per core
BS_REAL = BATCH // N_CORES  # 12500
P_OUT = 5 * LEN_PRED        # 125
K_IN = 2 * LEN_HIST         # 32


def _build_wc(vsx, vsy, asx, asy, GR, coef_G, len_pred):
    """Collapse the filter to W (32, 5L) and constant vector cvec (5L,)."""
    L = int(len_pred)
    H = np.zeros((2, 4)); H[0, 0] = 1.0; H[1, 2] = 1.0
    F = np.eye(4); F[0, 1] = DT; F[2, 3] = DT
    G = np.array([DT * DT / 2, DT, DT * DT / 2, DT])
    Id = np.eye(4)

    ax2 = float(asx[0]) ** 2
    ay2 = float(asy[0]) ** 2
    mx = np.array([1.0, 1.0, 0.0, 0.0]); my = 1.0 - mx
    scale = (ax2 * np.outer(mx, mx) + ay2 * np.outer(my, my)
             + np.outer(mx, my) + np.outer(my, mx))
    g = G * np.tanh(np.asarray(coef_G, np.float64))
    Q = np.outer(g, g) * scale
    R = np.outer(np.asarray(GR, np.float64), np.asarray(GR, np.float64))

    D0 = np.array([[1.0, 0.0], [-1.0 / DT, 0.0], [0.0, 1.0], [0.0, -1.0 / DT]])
    D1 = np.array([[0.0, 0.0], [1.0 / DT, 0.0], [0.0, 0.0], [0.0, 1.0 / DT]])
    P = np.diag([R[0, 0], float(vsx[0]) ** 2, R[1, 1], float(vsy[0]) ** 2])

    C = np.zeros((LEN_HIST, 4, 2))
    C[0] = D0; C[1] = D1
    for t in range(1, LEN_HIST):
        P = F @ P @ F.T + Q
        S = H @ P @ H.T + R
        K = P @ H.T @ np.linalg.inv(S)
        A = (Id - K @ H) @ F
        C = np.einsum('ij,tjk->tik', A, C)
        C[t] += K
        ImKH = Id - K @ H
        P = ImKH @ P @ ImKH.T + K @ R @ K.T

    W_dev = np.zeros((K_IN, 5 * L))
    cvec = np.zeros(5 * L)
    M = np.eye(4)
    for l in range(L):
        M = F @ M
        P = F @ P @ F.T + Q
        HFl = H @ M
        Wl = np.einsum('ij,tjk->itk', HFl, C)   # (2, T, 2)
        for ch in range(2):
            W_dev[:, l * 5 + ch] = Wl[ch].reshape(-1)
        Pout = H @ P @ H.T
        sx = np.sqrt(Pout[0, 0]); sy = np.sqrt(Pout[1, 1])
        cvec[l * 5 + 2] = sx
        cvec[l * 5 + 3] = sy
        cvec[l * 5 + 4] = (Pout[0, 1] + Pout[1, 0]) / (2.0 * sx * sy)
    return W_dev.astype(np.float32), cvec.astype(np.float32)


_NC_CACHE = {}


def _build_bass():
    import concourse.bass as bass
    import concourse.bacc as bacc
    import concourse.tile as tile
    from concourse import mybir

    nc = bacc.Bacc("TRN2", target_bir_lowering=False, debug=False,
                   num_devices=N_CORES)
    x = nc.declare_dram_parameter("x", [K_IN, BS], mybir.dt.float32, isOutput=False)
    w = nc.declare_dram_parameter("w", [K_IN, P_OUT], mybir.dt.float32, isOutput=False)
    cv = nc.declare_dram_parameter("cv", [P_OUT, 1], mybir.dt.float32, isOutput=False)
    out = nc.declare_dram_parameter("out", [P_OUT, BS], mybir.dt.float32, isOutput=True)

    with tile.TileContext(nc) as tc:
        with tc.tile_pool(name="singles", bufs=1) as singles, \
             tc.tile_pool(name="xin", bufs=4) as xin_pool, \
             tc.tile_pool(name="ps", bufs=4, space="PSUM") as psum_pool, \
             tc.tile_pool(name="op", bufs=4) as out_pool:
            w_tile = singles.tile([K_IN, P_OUT], mybir.dt.float32)
            nc.sync.dma_start(out=w_tile, in_=w[:, :])
            c_tile = singles.tile([P_OUT, 1], mybir.dt.float32)
            nc.sync.dma_start(out=c_tile, in_=cv[:, :])
            # Group 5 matmul tiles per DMA: 32x2560 in (320KB), 125x2560
            # out (1.25MB) — above the DMA efficiency knee.
            GRP = 5
            for g in range(NTILES // GRP):
                x_tile = xin_pool.tile([K_IN, GRP * NB], mybir.dt.float32)
                nc.sync.dma_start(
                    out=x_tile, in_=x[:, g * GRP * NB:(g + 1) * GRP * NB])
                o_tile = out_pool.tile([P_OUT, GRP * NB], mybir.dt.float32)
                for j in range(GRP):
                    ps = psum_pool.tile([P_OUT, NB], mybir.dt.float32)
                    nc.tensor.matmul(ps, w_tile,
                                     x_tile[:, j * NB:(j + 1) * NB],
                                     start=True, stop=True)
                    nc.scalar.activation(
                        out=o_tile[:, j * NB:(j + 1) * NB], in_=ps,
                        func=mybir.ActivationFunctionType.Identity,
                        bias=c_tile, scale=1.0,
                    )
                nc.sync.dma_start(
                    out=out[:, g * GRP * NB:(g + 1) * GRP * NB], in_=o_tile)
    nc.compile()
    return nc


def _get_nc():
    if "nc" not in _NC_CACHE:
        _NC_CACHE["nc"] = _build_bass()
    return _NC_CACHE["nc"]


def _run_device(hist_T, W, cvec, trace=False):
    from concourse.bass_utils import run_bass_kernel_spmd

    cv2 = cvec.reshape(P_OUT, 1)
    in_maps = []
    for c in range(N_CORES):
        shard = np.zeros((K_IN, BS), np.float32)
        shard[:, :BS_REAL] = hist_T[:, c * BS_REAL:(c + 1) * BS_REAL]
        in_maps.append({"x": shard, "w": W, "cv": cv2})
    res = run_bass_kernel_spmd(_get_nc(), in_maps, list(range(N_CORES)),
                               trace=trace)
    return res


def kernel(hist, velocity_std_x, velocity_std_y, acceleration_std_x,
           acceleration_std_y, GR, coef_G, len_pred):
    hist = np.asarray(hist, np.float32)
    L = int(len_pred)
    W, cvec = _build_wc(velocity_std_x, velocity_std_y, acceleration_std_x,
                        acceleration_std_y, GR, coef_G, L)
    T, B, _ = hist.shape
    hist_T = np.ascontiguousarray(hist.transpose(0, 2, 1)).reshape(2 * T, B)

    if L != LEN_PRED or B != BATCH or T != LEN_HIST:
        # shape surprise: fall back to exact host math
        out_flat = W.T @ hist_T + cvec[:, None]
        return np.ascontiguousarray(
            out_flat.reshape(L, 5, B).transpose(0, 2, 1)).astype(np.float32)

    res = _run_device(hist_T, W, cvec)
    out = np.empty((LEN_PRED, B, 5), np.float32)
    for c in range(N_CORES):
        oc = res.results[c]["out"][:, :BS_REAL]          # (125, 12500)
        out[:, c * BS_REAL:(c + 1) * BS_REAL, :] = (
            oc.reshape(LEN_PRED, 5, BS_REAL).transpose(0, 2, 1))
    return out



# revision 2
# speedup vs baseline: 3.4476x; 3.4476x over previous
"""Kalman CV filter (nn_KalmanCV) — Trainium2 Bass kernel, 8-core data parallel.

Math: the covariance P (and thus the Kalman gains K_t and the output
channels sx/sy/rho) is batch-independent — it depends only on the scalar
inputs. The whole per-batch computation collapses to a linear map over
the 32 history scalars:

    out[l, b, ch<2] = sum_{t,ci} W[t*2+ci, l*2+ch] * hist[t, b, ci]
    out[l, b, ch>=2] = const[l, ch]          (sx, sy, rho — host-filled)

Device kernel per core (all bf16 I/O):
  - 2-block-diagonal weight packing: W2 = blockdiag(Wmu, Wmu) of shape
    (64, 128-padded), so each 512-col matmul tile processes TWO batch
    chunks at once (contraction 64, output partitions 100). Halves the
    columns streamed through the PE.
  - Only the 50 batch-dependent output rows (mu_x/mu_y per step) are
    computed and DMA'd out; the 75 constant rows never touch the device.
  - Output DMA is split into one call per 512-col tile, issued from
    multiple queues, because SBUF->HBM calls pin to a single SDMA engine
    (~25 GB/s each); splitting spreads them across engines.
"""
import numpy as np
import ml_dtypes

DT = 0.2
LEN_HIST = 16
LEN_PRED = 25
BATCH = 100000

N_CORES = 8
TILE = 512                  # matmul free size = one PSUM bank of f32
NT = 13                     # tiles per core
COLS = TILE * NT            # 6656 padded columns per core
BLK = BATCH // N_CORES // 2 # 6250 real batch per block (2 blocks/core)
NT_A = 7                    # tiles covered by first input DMA
COLS_A = NT_A * TILE        # 3584
COLS_B = COLS - COLS_A      # 3072
K2 = 64                     # packed contraction dim (2 x 32)
M_OUT = 100                 # 2 blocks x 50 mu rows
M_PAD = 128                 # weight free size padded for fast weight load

BF16 = ml_dtypes.bfloat16


def _build_wc(vsx, vsy, asx, asy, GR, coef_G, len_pred):
    """Collapse the filter to W (32, 5L) and constant vector cvec (5L,)."""
    L = int(len_pred)
    H = np.zeros((2, 4)); H[0, 0] = 1.0; H[1, 2] = 1.0
    F = np.eye(4); F[0, 1] = DT; F[2, 3] = DT
    G = np.array([DT * DT / 2, DT, DT * DT / 2, DT])
    Id = np.eye(4)

    ax2 = float(asx[0]) ** 2
    ay2 = float(asy[0]) ** 2
    mx = np.array([1.0, 1.0, 0.0, 0.0]); my = 1.0 - mx
    scale = (ax2 * np.outer(mx, mx) + ay2 * np.outer(my, my)
             + np.outer(mx, my) + np.outer(my, mx))
    g = G * np.tanh(np.asarray(coef_G, np.float64))
    Q = np.outer(g, g) * scale
    R = np.outer(np.asarray(GR, np.float64), np.asarray(GR, np.float64))

    D0 = np.array([[1.0, 0.0], [-1.0 / DT, 0.0], [0.0, 1.0], [0.0, -1.0 / DT]])
    D1 = np.array([[0.0, 0.0], [1.0 / DT, 0.0], [0.0, 0.0], [0.0, 1.0 / DT]])
    P = np.diag([R[0, 0], float(vsx[0]) ** 2, R[1, 1], float(vsy[0]) ** 2])

    C = np.zeros((LEN_HIST, 4, 2))
    C[0] = D0; C[1] = D1
    for t in range(1, LEN_HIST):
        P = F @ P @ F.T + Q
        S = H @ P @ H.T + R
        K = P @ H.T @ np.linalg.inv(S)
        A = (Id - K @ H) @ F
        C = np.einsum('ij,tjk->tik', A, C)
        C[t] += K
        ImKH = Id - K @ H
        P = ImKH @ P @ ImKH.T + K @ R @ K.T

    W_dev = np.zeros((2 * LEN_HIST, 5 * L))
    cvec = np.zeros(5 * L)
    M = np.eye(4)
    for l in range(L):
        M = F @ M
        P = F @ P @ F.T + Q
        HFl = H @ M
        Wl = np.einsum('ij,tjk->itk', HFl, C)   # (2, T, 2)
        for ch in range(2):
            W_dev[:, l * 5 + ch] = Wl[ch].reshape(-1)
        Pout = H @ P @ H.T
        sx = np.sqrt(Pout[0, 0]); sy = np.sqrt(Pout[1, 1])
        cvec[l * 5 + 2] = sx
        cvec[l * 5 + 3] = sy
        cvec[l * 5 + 4] = (Pout[0, 1] + Pout[1, 0]) / (2.0 * sx * sy)
    return W_dev, cvec


_NC_CACHE = {}


def _build_bass():
    import concourse.bass as bass
    import concourse.bacc as bacc
    import concourse.tile as tile
    from concourse import mybir

    nc = bacc.Bacc("TRN2", target_bir_lowering=False, debug=False,
                   num_devices=N_CORES)
    xa = nc.declare_dram_parameter("xa", [K2, COLS_A], mybir.dt.bfloat16, isOutput=False)
    xb = nc.declare_dram_parameter("xb", [K2, COLS_B], mybir.dt.bfloat16, isOutput=False)
    w = nc.declare_dram_parameter("w", [K2, M_PAD], mybir.dt.bfloat16, isOutput=False)
    out = nc.declare_dram_parameter("out", [M_OUT, COLS], mybir.dt.bfloat16, isOutput=True)

    with tile.TileContext(nc) as tc:
        with tc.tile_pool(name="singles", bufs=1) as singles, \
             tc.tile_pool(name="ps", bufs=8, space="PSUM") as psum_pool, \
             tc.tile_pool(name="op", bufs=NT) as out_pool:
            w_tile = singles.tile([K2, M_PAD], mybir.dt.bfloat16)
            nc.sync.dma_start(out=w_tile, in_=w[:, :])
            xa_tile = singles.tile([K2, COLS_A], mybir.dt.bfloat16)
            nc.sync.dma_start(out=xa_tile, in_=xa[:, :])
            xb_tile = singles.tile([K2, COLS_B], mybir.dt.bfloat16)
            nc.sync.dma_start(out=xb_tile, in_=xb[:, :])

            for j in range(NT):
                if j < NT_A:
                    x_sl = xa_tile[:, j * TILE:(j + 1) * TILE]
                else:
                    x_sl = xb_tile[:, (j - NT_A) * TILE:(j - NT_A + 1) * TILE]
                ps = psum_pool.tile([M_PAD, TILE], mybir.dt.float32)
                nc.tensor.matmul(ps, w_tile, x_sl, start=True, stop=True)
                o_tile = out_pool.tile([M_OUT, TILE], mybir.dt.bfloat16)
                if j % 13 in (0, 3, 6, 9, 12):
                    nc.scalar.copy(out=o_tile, in_=ps[:M_OUT, :])
                else:
                    nc.vector.tensor_scalar_add(o_tile, ps[:M_OUT, :], 0.0)
                issuer = (nc.sync, nc.scalar, nc.gpsimd)[j % 3]
                issuer.dma_start(out=out[:, j * TILE:(j + 1) * TILE], in_=o_tile)
    nc.compile()
    return nc


def _get_nc():
    if "nc" not in _NC_CACHE:
        _NC_CACHE["nc"] = _build_bass()
    return _NC_CACHE["nc"]


def _pack_inputs(hist_T_bf, W2):
    """Per-core input dicts: block-packed (64, COLS) bf16 halves."""
    per_core = BATCH // N_CORES
    in_maps = []
    for c in range(N_CORES):
        x2 = np.zeros((K2, COLS), dtype=BF16)
        base = c * per_core
        x2[:32, :BLK] = hist_T_bf[:, base:base + BLK]
        x2[32:, :BLK] = hist_T_bf[:, base + BLK:base + 2 * BLK]
        in_maps.append({"xa": np.ascontiguousarray(x2[:, :COLS_A]),
                        "xb": np.ascontiguousarray(x2[:, COLS_A:]),
                        "w": W2})
    return in_maps


def _run_device(hist_T_bf, W2, trace=False):
    from concourse.bass_utils import run_bass_kernel_spmd
    in_maps = _pack_inputs(hist_T_bf, W2)
    return run_bass_kernel_spmd(_get_nc(), in_maps, list(range(N_CORES)),
                                trace=trace)


def _pack_w(W_dev, L):
    """(32, 5L) f64 -> blockdiag-packed (64, M_PAD) bf16, mu rows only."""
    Wmu = np.zeros((32, 2 * L))
    for l in range(L):
        Wmu[:, 2 * l] = W_dev[:, 5 * l]
        Wmu[:, 2 * l + 1] = W_dev[:, 5 * l + 1]
    W2 = np.zeros((K2, M_PAD), dtype=BF16)
    W2[:32, :2 * L] = Wmu.astype(BF16)
    W2[32:, 2 * L:4 * L] = Wmu.astype(BF16)
    return W2


def kernel(hist, velocity_std_x, velocity_std_y, acceleration_std_x,
           acceleration_std_y, GR, coef_G, len_pred):
    hist = np.asarray(hist, np.float32)
    L = int(len_pred)
    W_dev, cvec = _build_wc(velocity_std_x, velocity_std_y, acceleration_std_x,
                            acceleration_std_y, GR, coef_G, L)
    T, B, _ = hist.shape
    hist_T = np.ascontiguousarray(hist.transpose(0, 2, 1)).reshape(2 * T, B)

    if L != LEN_PRED or B != BATCH or T != LEN_HIST:
        # shape surprise: fall back to exact host math
        out_flat = W_dev.astype(np.float32).T @ hist_T \
            + cvec.astype(np.float32)[:, None]
        return np.ascontiguousarray(
            out_flat.reshape(L, 5, B).transpose(0, 2, 1)).astype(np.float32)

    W2 = _pack_w(W_dev, L)
    hist_T_bf = hist_T.astype(BF16)
    res = _run_device(hist_T_bf, W2)

    per_core = B // N_CORES
    out = np.empty((L, B, 5), np.float32)
    # constant channels: sx, sy, rho
    out[:, :, 2] = cvec[2::5].astype(np.float32)[:, None]
    out[:, :, 3] = cvec[3::5].astype(np.float32)[:, None]
    out[:, :, 4] = cvec[4::5].astype(np.float32)[:, None]
    for c in range(N_CORES):
        oc = np.asarray(res.results[c]["out"]).astype(np.float32)  # (100, COLS)
        base = c * per_core
        # block A: rows 0:50 -> (25, 2, BLK) -> (25, BLK, 2)
        out[:, base:base + BLK, :2] = (
            oc[:50, :BLK].reshape(L, 2, BLK).transpose(0, 2, 1))
        out[:, base + BLK:base + 2 * BLK, :2] = (
            oc[50:100, :BLK].reshape(L, 2, BLK).transpose(0, 2, 1))
    return out
